# revision 13
# baseline (speedup 1.0000x reference)
"""Multi-head attention (N=2048, D=1024, H=16) on 8 TRN2 NeuronCores.

Sharding: tensor-parallel over heads (2 heads / core). x is replicated,
each core computes QKV / scores / softmax / PV / out-proj for its 2
heads, producing a partial (N, D) projection output in fp16. The
all-reduce over cores is the host-side f64 sum of the 8 partials
(+ b_proj), cast back to f32.

Precision/speed mix (softmax-weight noise passes through ~1:1, so the
logits path stays bf16-accurate; only noise that averages out rides fp8):
  QKV(q,k) bf16 matmuls (exact logits);  QKV(v) fp8e4m3 DoubleRow with
        16x host-scaled weights (v noise averages out in PV; the 1/16
        is folded into w_proj).
  q/k   optionally stored fp8 pair-packed [64, 2, n] so scores can run
        DoubleRow with Ki=32 (QK_FP8 knob; ~1.3% logit noise).
  V.T   --PE transpose (bf16)--> v_sb [seq, mc, 2*(DH+1)] with ones
        columns for the PV rowsum.
  exp   bf16 es, split between ACT (true exp) and DVE (Schraudolph:
        one tensor_scalar into uint16 = bf16 bit pattern). A greedy
        balancer assigns every exp/drain op to the less-loaded engine;
        EXP_DVE_MAX caps how many exps may take the (noisier) DVE path.
  PV    bf16 with rowsum ones-column.
  div   DVE reciprocal + DMA round-trip partition-broadcast via DRAM
        + DVE muls.
  proj  bf16 matmuls; f16 drains; one DMA per 512-row block.
"""

import os
import sys

import numpy as np

for _p in ("/opt/trn_rl_repo",):
    if os.path.isdir(_p) and _p not in sys.path:
        sys.path.insert(0, _p)

N, D, H = 2048, 1024, 16
DH = D // H                 # 64
NCORES = 8
HPC = H // NCORES           # 2 heads per core
P = 128
SCALE = 1.0 / DH ** 0.5

D_CHUNKS = D // P           # 8
G_CHUNKS = D_CHUNKS // 2    # 4 d-chunk pairs (DoubleRow v)
WSCALE = 16.0               # host-side fp8 v-weight scale (undone in w_proj)

NB = int(os.environ.get("ATTN_NB", "512"))                 # query-block size
SPS_BUFS = int(os.environ.get("ATTN_SPS_BUFS", "2"))
ES_BUFS = int(os.environ.get("ATTN_ES_BUFS", "6"))
QK_FP8 = os.environ.get("ATTN_QK_FP8", "0") == "1"         # DoubleRow scores
EXP_DVE_MAX = int(os.environ.get("ATTN_EXP_DVE_MAX", "32"))
WARMUP_MM = int(os.environ.get("ATTN_WARMUP", "26"))

# exp(s * SCALE) from psum scores; bf16 Schraudolph constants
SCH_A = 128.0 * np.log2(np.e) * SCALE                      # uint16 mult
SCH_B = float(os.environ.get("ATTN_SCH_B", "16249.1"))     # uint16 bias


def _build_nc(n=N, nb=NB):
    """Build the per-core Bass module (SPMD: identical program, per-core data)."""
    import concourse.bass as bass  # noqa: F401
    import concourse.mybir as mybir
    import concourse.tile as tile
    from concourse import bacc
    from concourse.masks import make_identity

    f32 = mybir.dt.float32
    f32r = mybir.dt.float32r
    bf16 = mybir.dt.bfloat16
    f16 = mybir.dt.float16
    fp8 = mybir.dt.float8e4
    u16 = mybir.dt.uint16
    AF = mybir.ActivationFunctionType
    DR = mybir.MatmulPerfMode.DoubleRow
    ALU = mybir.AluOpType

    m_chunks = n // P
    n_blocks = n // nb
    qk_dt = fp8 if QK_FP8 else bf16

    nc = bacc.Bacc(
        "TRN2",
        target_bir_lowering=False,
        debug=False,
        enable_asserts=True,
        num_devices=NCORES,
    )

    xT_d = nc.dram_tensor("xT", (P, D_CHUNKS, n), bf16, kind="ExternalInput")
    wqkv_d = nc.dram_tensor("wqkv", (P, 3, D_CHUNKS, P), bf16, kind="ExternalInput")
    wp_d = nc.dram_tensor("wpT", (P, D), bf16, kind="ExternalInput")
    bqkv_d = nc.dram_tensor("bqkv", (P, 3), f32, kind="ExternalInput")
    out_d = nc.dram_tensor("out_part", (n, D), f16, kind="ExternalOutput")

    # ---- static ACT/DVE load balancer ----
    load = {"act": 0.0, "dve": 0.0}
    dve_exps = [0]

    def pick(act_cost, dve_cost, forced=None):
        if forced is None:
            eng = "act" if load["act"] + act_cost <= load["dve"] + dve_cost else "dve"
        else:
            eng = forced
        load[eng] += act_cost if eng == "act" else dve_cost
        return eng

    with tile.TileContext(nc) as tc:
        with (
            tc.tile_pool(name="consts", bufs=1) as consts,
            tc.tile_pool(name="xpool", bufs=1) as xpool,
            tc.tile_pool(name="qkpool", bufs=1) as qkpool,
        ):
            # ---- inputs ----
            wqkv_sb = consts.tile([P, 3, D_CHUNKS, P], bf16)
            wp_sb = consts.tile([P, D], bf16)
            bqkv_sb = consts.tile([P, 3], f32)
            xT_sb = xpool.tile([P, D_CHUNKS, n], bf16)

            qw = min(512, n)
            n_j = n // qw
            nc.sync.dma_start(bqkv_sb[:], bqkv_d.ap())
            # k first: it gates the first scores matmul
            nc.sync.dma_start(wqkv_sb[:, 1], wqkv_d.ap()[:, 1])

            def x_piece(j):
                nc.sync.dma_start(
                    xT_sb[:, :, j * qw:(j + 1) * qw],
                    xT_d.ap()[:, :, j * qw:(j + 1) * qw],
                )

            x_piece(0)
            nc.sync.dma_start(wqkv_sb[:, 0], wqkv_d.ap()[:, 0])
            nc.sync.dma_start(wqkv_sb[:, 2], wqkv_d.ap()[:, 2])
            for j in range(1, n_j):
                x_piece(j)
            nc.sync.dma_start(wp_sb[:], wp_d.ap())

            ident = consts.tile([P, P], bf16)
            make_identity(nc, ident[:])
            # ones row for the reciprocal partition-broadcast matmul
            ones_f32 = consts.tile([P, DH], f32)
            nc.gpsimd.memset(ones_f32[0:1, :], 1.0)
            ones_sb = consts.tile([P, DH], f32r)
            nc.vector.tensor_copy(ones_sb[0:1, :], ones_f32[0:1, :])

            # ---- persistent activations ----
            if QK_FP8:
                # pair-packed for DoubleRow scores: [64, 2, n]
                #   partition p: head = p//32, dh = (p%32) + 32*slot
                qT_sb = qkpool.tile([DH, 2, n], fp8)
                kT_sb = qkpool.tile([DH, 2, n], fp8)
            else:
                qT_sb = qkpool.tile([P, n], bf16)
                kT_sb = qkpool.tile([P, n], bf16)
            vT_sb = qkpool.tile([P, n], bf16)       # feature-major V.T (16x)
            # PV V layout: [seq-in-chunk, mc, [V_h0|1|V_h1|1]]
            v_sb = qkpool.tile([P, m_chunks, 2 * (DH + 1)], bf16)
            nc.gpsimd.memset(v_sb[:, :, DH:DH + 1], 1.0)
            nc.gpsimd.memset(v_sb[:, :, 2 * DH + 1:2 * DH + 2], 1.0)

            # ===== PSUM pools (8 banks):
            #   sps:  scores [128, 2*NB] f32 -> 2 banks x SPS_BUFS
            #   pvps: PV acc [128, 2*NB] f32 -> 2 banks, single buffered
            #   auxp: [128, 512] f32 1-bank tiles x2 (QKV acc / transposes / proj)
            sps = tc.alloc_tile_pool(name="sps", bufs=SPS_BUFS, space="PSUM")
            pvps = tc.alloc_tile_pool(name="pvps", bufs=1, space="PSUM")
            auxp = tc.alloc_tile_pool(name="auxp", bufs=2, space="PSUM")

            # ---- engine-dispatched op emitters ----
            def drain(dst, src, bias=None, act_c=612.0, dve_c=658.0, forced=None):
                """PSUM->SBUF copy (+per-partition bias) on ACT or DVE."""
                eng = pick(act_c, dve_c, forced)
                if eng == "act":
                    if bias is None:
                        nc.scalar.copy(dst, src)
                    else:
                        nc.scalar.activation(dst, src, AF.Identity, bias=bias)
                else:
                    if bias is None:
                        nc.vector.tensor_copy(dst, src)
                    else:
                        nc.vector.tensor_scalar(dst, src, bias, None, ALU.add)

            def exp_op(dst_bf16, src_ps):
                if dve_exps[0] < EXP_DVE_MAX:
                    eng = pick(1038.0, 1190.0)
                else:
                    eng = pick(1038.0, 1190.0, forced="act")
                if eng == "act":
                    nc.scalar.activation(dst_bf16, src_ps, AF.Exp, scale=SCALE)
                else:
                    dve_exps[0] += 1
                    nc.vector.tensor_scalar(
                        dst_bf16.bitcast(u16), src_ps, float(SCH_A), float(SCH_B),
                        ALU.mult, ALU.add,
                    )

            # ---- PE warmup: burn the p-state ramp while DMAs land ----
            if WARMUP_MM:
                wps = auxp.tile([P, P], bf16, tag="aux", name="warm")
                for i in range(WARMUP_MM):
                    nc.tensor.transpose(wps[:], ident[:], ident[:])

            # ================= QKV ===========================================
            def qkv_j(j, parts):
                jsl = slice(j * qw, (j + 1) * qw)
                for part in parts:
                    ps = auxp.tile([P, qw], f32, tag="aux", name=f"qkv_{part}_{j}")
                    if part == 2:
                        for o in range(D_CHUNKS):
                            nc.tensor.matmul(
                                ps[:], wqkv_sb[:, part, o], xT_sb[:, o, jsl],
                                start=(o == 0), stop=(o == D_CHUNKS - 1),
                            )
                        drain(vT_sb[:, jsl], ps[:],
                              bias=bqkv_sb[:, 2:3])
                        # V.T -> v_sb via PE transpose per m-chunk
                        for mc in range(j * qw // P, (j + 1) * qw // P):
                            tp = auxp.tile([P, P], bf16, tag="aux", name=f"tp_{mc}")
                            nc.tensor.transpose(
                                tp[:], vT_sb[:, mc * P:(mc + 1) * P], ident[:]
                            )
                            # both heads in one strided copy [128, 2, 64]
                            drain(
                                v_sb[:, mc, :].rearrange(
                                    "p (h c) -> p h c", h=2
                                )[:, :, 0:DH],
                                tp[:].rearrange("p (h c) -> p h c", h=2),
                                act_c=292.0, dve_c=258.0,
                            )
                    else:
                        for o in range(D_CHUNKS):
                            nc.tensor.matmul(
                                ps[:], wqkv_sb[:, part, o], xT_sb[:, o, jsl],
                                start=(o == 0), stop=(o == D_CHUNKS - 1),
                            )
                        dst = qT_sb if part == 0 else kT_sb
                        if QK_FP8:
                            # pair-split drains into [64, 2, n] layout
                            for s in range(2):
                                drain(
                                    dst[:, s, jsl],
                                    ps[s * DH:(s + 1) * DH, :],
                                    bias=bqkv_sb[s * DH:(s + 1) * DH, part:part + 1],
                                )
                        else:
                            drain(dst[:, jsl], ps[:], bias=bqkv_sb[:, part:part + 1])

            # ================= attention =====================================
            with (
                tc.tile_pool(name="espool", bufs=ES_BUFS) as espool,
                tc.tile_pool(name="opool", bufs=2) as opool,
                tc.tile_pool(name="outpool", bufs=2) as outpool,
                tc.tile_pool(name="rpool", bufs=2) as rpool,
            ):
                def attn_sc(b, row0, nbb, mcs):
                    nsl = slice(row0, row0 + nbb)
                    out = []
                    for mc in mcs:
                        s_ps = sps.tile([P, 2 * nbb], f32, tag="s",
                                        name=f"s_ps_{b}_{mc}")
                        for h in range(HPC):
                            if QK_FP8:
                                nc.tensor.matmul(
                                    s_ps[:, h * nbb:(h + 1) * nbb],
                                    kT_sb[32 * h:32 * (h + 1), :,
                                          mc * P:(mc + 1) * P],
                                    qT_sb[32 * h:32 * (h + 1), :, nsl],
                                    perf_mode=DR,
                                )
                            else:
                                nc.tensor.matmul(
                                    s_ps[:, h * nbb:(h + 1) * nbb],
                                    kT_sb[h * DH:(h + 1) * DH,
                                          mc * P:(mc + 1) * P],
                                    qT_sb[h * DH:(h + 1) * DH, nsl],
                                    tile_position=(h * DH, 0),
                                )
                        es = espool.tile([P, 2 * nbb], bf16, tag="es",
                                         name=f"es_{b}_{mc}")
                        exp_op(es[:], s_ps[:])
                        out.append((mc, es))
                    return out

                def attn_pv(nbb, pvs, mc_es):
                    for mc, es in mc_es:
                        for h in range(HPC):
                            nc.tensor.matmul(
                                pvs[0:DH + 1, h * nbb:(h + 1) * nbb],
                                v_sb[:, mc, h * (DH + 1):(h + 1) * (DH + 1)],
                                es[:, h * nbb:(h + 1) * nbb],
                                start=(mc == 0),
                                stop=(mc == m_chunks - 1),
                            )

                def attn_mc_group(b, row0, nbb, pvs, mcs):
                    attn_pv(nbb, pvs, attn_sc(b, row0, nbb, mcs))

                def division(b, nbb, pvs):
                    # O.T = O'.T / rowsum, heads stacked on partitions.
                    rt = rpool.tile([P, HPC * nbb], f32r, tag="recip",
                                    name=f"rt_{b}")
                    rb = rpool.tile([DH, HPC * nbb], f32, tag="rbcast",
                                    name=f"rb_{b}")
                    oT = opool.tile([P, nbb], bf16, tag="oT", name=f"oT_{b}")
                    with nc.allow_low_precision(reason="f32r recip"):
                        nc.vector.reciprocal(rt[0:1, :], pvs[DH:DH + 1, :])
                    load["dve"] += 1190.0
                    for h in range(HPC):
                        hs = slice(h * nbb, (h + 1) * nbb)
                        rb_ps = auxp.tile([P, nbb], f32, tag="aux",
                                          name=f"rb_{b}_{h}")
                        nc.tensor.matmul(rb_ps[0:DH, :], ones_sb[0:1, :],
                                         rt[0:1, hs])
                        drain(rb[:, hs], rb_ps[0:DH, :])
                        nc.vector.tensor_mul(
                            oT[h * DH:(h + 1) * DH, :],
                            pvs[0:DH, hs],
                            rb[:, hs],
                        )
                        load["dve"] += 658.0
                    return oT

                def projection(b, row0, nbb, oT, last=False):
                    nch = nbb // P
                    out_sb = outpool.tile([P, nch, D], f16, tag="out",
                                          name=f"out_{b}")
                    for j in range(nch):
                        for half in range(D // 512):
                            pp = auxp.tile([P, 512], f32, tag="aux",
                                           name=f"pp_{b}_{j}_{half}")
                            nc.tensor.matmul(
                                pp[:],
                                oT[:, j * P:(j + 1) * P],
                                wp_sb[:, half * 512:(half + 1) * 512],
                            )
                            drain(out_sb[:, j, half * 512:(half + 1) * 512], pp[:])
                    nc.sync.dma_start(
                        out_d.ap()[row0:row0 + nbb, :].rearrange(
                            "(c p) d -> p c d", p=P
                        ),
                        out_sb[:],
                    )

                blocks = [nb] * n_blocks
                pending = None   # (b, row0, nbb, oT) awaiting projection
                row0 = 0
                for b, nbb in enumerate(blocks):
                    pvs = pvps.tile([P, HPC * nbb], f32, tag="pv", name=f"pv_{b}")
                    if b == 0:
                        # fine interleave with QKV j-sweeps
                        mcs_per_j = qw // P
                        for j in range(n_j):
                            qkv_j(j, parts=(1, 0) if j == 0 else (1,))
                            mc_es = attn_sc(b, row0, nbb,
                                            range(j * mcs_per_j,
                                                  (j + 1) * mcs_per_j))
                            qkv_j(j, parts=(2,))
                            attn_pv(nbb, pvs, mc_es)
                        if n_j > 1:
                            qkv_j(1, parts=(0,))
                    else:
                        split = min(4, m_chunks)
                        attn_mc_group(b, row0, nbb, pvs, range(0, split))
                        if pending is not None:
                            projection(*pending)
                            pending = None
                        if b + 1 < n_j:
                            qkv_j(b + 1, parts=(0,))
                        attn_mc_group(b, row0, nbb, pvs, range(split, m_chunks))
                    oT = division(b, nbb, pvs)
                    if pending is not None:
                        projection(*pending)
                        pending = None
                    pending = (b, row0, nbb, oT)
                    row0 += nbb
                projection(*pending, last=True)

            auxp.release()
            pvps.release()
            sps.release()

    nc.compile()
    return nc


def _host_prep(x, w_qkv, b_qkv, w_proj, n=N):
    """Per-core input maps (dtypes match the DRAM tensor declarations)."""
    import ml_dtypes

    fp8 = ml_dtypes.float8_e4m3
    bf = ml_dtypes.bfloat16

    xT = np.ascontiguousarray(
        x.T.reshape(D_CHUNKS, P, n).transpose(1, 0, 2).astype(bf)
    )

    if QK_FP8:
        # q/k feature order for PSUM partition j:
        #   slot s = j // 64, head h = (j % 64) // 32, dh = (j % 32) + 32*s
        j_idx = np.arange(P)
        s_idx, jj = j_idx // DH, j_idx % DH
        qk_feat = (jj // 32) * DH + (jj % 32) + 32 * s_idx
    else:
        qk_feat = np.arange(P)

    in_maps = []
    for c in range(NCORES):
        parts = []
        border = []
        for part in range(3):
            rows = w_qkv[part * D + c * P:part * D + (c + 1) * P, :]
            feat = qk_feat if part != 2 else np.arange(P)
            wperm = rows[feat, :]                          # [128 feat, D]
            # lhsT layout [p, o, feat]
            parts.append(wperm.T.reshape(D_CHUNKS, P, P).transpose(1, 0, 2))
            border.append(b_qkv[part * D + c * P:part * D + (c + 1) * P][feat])
        wqkv = np.ascontiguousarray(np.stack(parts, axis=1).astype(bf))
        bqkv = np.ascontiguousarray(np.stack(border, axis=1).astype(np.float32))
        wpT = np.ascontiguousarray(w_proj[:, c * P:(c + 1) * P].T.astype(bf))
        in_maps.append({"xT": xT, "wqkv": wqkv, "wpT": wpT, "bqkv": bqkv})
    return in_maps


_NC_CACHE = {}


def run(x, w_qkv, b_qkv, w_proj, b_proj, trace=False, n=N, nb=None, **spmd_kwargs):
    from concourse.bass_utils import run_bass_kernel_spmd

    if nb is None:
        nb = NB
    key = (n, nb, SPS_BUFS, ES_BUFS, QK_FP8, EXP_DVE_MAX)
    if key not in _NC_CACHE:
        _NC_CACHE[key] = _build_nc(n=n, nb=nb)
    nc = _NC_CACHE[key]

    in_maps = _host_prep(
        np.asarray(x), np.asarray(w_qkv), np.asarray(b_qkv), np.asarray(w_proj), n=n
    )
    results = run_bass_kernel_spmd(
        nc, in_maps, core_ids=list(range(NCORES)), trace=trace, **spmd_kwargs
    )
    acc = np.zeros((n, D), dtype=np.float64)
    for c in range(NCORES):
        acc += results.results[c]["out_part"].astype(np.float64)
    acc += np.asarray(b_proj).astype(np.float64)
    return acc.astype(np.float32), results


def kernel(x, w_qkv, b_qkv, w_proj, b_proj):
    out, _ = run(x, w_qkv, b_qkv, w_proj, b_proj, trace=False)
    return out


# revision 14
# speedup vs baseline: 1.0147x; 1.0147x over previous
"""Multi-head attention (N=2048, D=1024, H=16) on 8 TRN2 NeuronCores.

Sharding: tensor-parallel over heads (2 heads / core). x is replicated,
each core computes QKV / scores / softmax / PV / out-proj for its 2
heads, producing a partial (N, D) projection output in fp16. The
all-reduce over cores is the host-side f64 sum of the 8 partials
(+ b_proj), cast back to f32.

Precision/speed mix (softmax-weight noise passes through ~1:1, so the
logits path stays bf16-accurate; only noise that averages out rides fp8):
  QKV(q,k) bf16 matmuls (exact logits);  QKV(v) fp8e4m3 DoubleRow with
        16x host-scaled weights (v noise averages out in PV; the 1/16
        is folded into w_proj).
  q/k   optionally stored fp8 pair-packed [64, 2, n] so scores can run
        DoubleRow with Ki=32 (QK_FP8 knob; ~1.3% logit noise).
  V.T   --PE transpose (bf16)--> v_sb [seq, mc, 2*(DH+1)] with ones
        columns for the PV rowsum.
  exp   bf16 es, split between ACT (true exp) and DVE (Schraudolph:
        one tensor_scalar into uint16 = bf16 bit pattern). A greedy
        balancer assigns every exp/drain op to the less-loaded engine;
        EXP_DVE_MAX caps how many exps may take the (noisier) DVE path.
  PV    bf16 with rowsum ones-column.
  div   DVE reciprocal + DMA round-trip partition-broadcast via DRAM
        + DVE muls.
  proj  bf16 matmuls; f16 drains; one DMA per 512-row block.
"""

import os
import sys

import numpy as np

for _p in ("/opt/trn_rl_repo",):
    if os.path.isdir(_p) and _p not in sys.path:
        sys.path.insert(0, _p)

N, D, H = 2048, 1024, 16
DH = D // H                 # 64
NCORES = 8
HPC = H // NCORES           # 2 heads per core
P = 128
SCALE = 1.0 / DH ** 0.5

D_CHUNKS = D // P           # 8
G_CHUNKS = D_CHUNKS // 2    # 4 d-chunk pairs (DoubleRow v)
WSCALE = 16.0               # host-side fp8 v-weight scale (undone in w_proj)

NB = int(os.environ.get("ATTN_NB", "512"))                 # query-block size
SPS_BUFS = int(os.environ.get("ATTN_SPS_BUFS", "4"))
ES_BUFS = int(os.environ.get("ATTN_ES_BUFS", "10"))
QK_FP8 = os.environ.get("ATTN_QK_FP8", "0") == "1"         # DoubleRow scores
EXP_DVE_MAX = int(os.environ.get("ATTN_EXP_DVE_MAX", "64"))
WARMUP_MM = int(os.environ.get("ATTN_WARMUP", "26"))

# exp(s * SCALE) from psum scores; bf16 Schraudolph constants
SCH_A = 128.0 * np.log2(np.e) * SCALE                      # uint16 mult
SCH_B = float(os.environ.get("ATTN_SCH_B", "16249.1"))     # uint16 bias


def _build_nc(n=N, nb=NB):
    """Build the per-core Bass module (SPMD: identical program, per-core data)."""
    import concourse.bass as bass  # noqa: F401
    import concourse.mybir as mybir
    import concourse.tile as tile
    from concourse import bacc
    from concourse.masks import make_identity

    f32 = mybir.dt.float32
    f32r = mybir.dt.float32r
    bf16 = mybir.dt.bfloat16
    f16 = mybir.dt.float16
    fp8 = mybir.dt.float8e4
    u16 = mybir.dt.uint16
    AF = mybir.ActivationFunctionType
    DR = mybir.MatmulPerfMode.DoubleRow
    ALU = mybir.AluOpType

    m_chunks = n // P
    n_blocks = n // nb
    qk_dt = fp8 if QK_FP8 else bf16

    nc = bacc.Bacc(
        "TRN2",
        target_bir_lowering=False,
        debug=False,
        enable_asserts=True,
        num_devices=NCORES,
    )

    xT_d = nc.dram_tensor("xT", (P, D_CHUNKS, n), bf16, kind="ExternalInput")
    wqkv_d = nc.dram_tensor("wqkv", (P, 3, D_CHUNKS, P), bf16, kind="ExternalInput")
    wp_d = nc.dram_tensor("wpT", (P, D), bf16, kind="ExternalInput")
    bqkv_d = nc.dram_tensor("bqkv", (P, 3), f32, kind="ExternalInput")
    out_d = nc.dram_tensor("out_part", (n, D), f16, kind="ExternalOutput")

    # ---- static ACT/DVE load balancer ----
    load = {"act": 0.0, "dve": 0.0}
    dve_exps = [0]

    def pick(act_cost, dve_cost, forced=None):
        if forced is None:
            eng = "act" if load["act"] + act_cost <= load["dve"] + dve_cost else "dve"
        else:
            eng = forced
        load[eng] += act_cost if eng == "act" else dve_cost
        return eng

    with tile.TileContext(nc) as tc:
        with (
            tc.tile_pool(name="consts", bufs=1) as consts,
            tc.tile_pool(name="xpool", bufs=1) as xpool,
            tc.tile_pool(name="qkpool", bufs=1) as qkpool,
        ):
            # ---- inputs ----
            wqkv_sb = consts.tile([P, 3, D_CHUNKS, P], bf16)
            wp_sb = consts.tile([P, D], bf16)
            bqkv_sb = consts.tile([P, 3], f32)
            xT_sb = xpool.tile([P, D_CHUNKS, n], bf16)

            qw = min(512, n)
            n_j = n // qw
            nc.sync.dma_start(bqkv_sb[:], bqkv_d.ap())
            # k first: it gates the first scores matmul
            nc.sync.dma_start(wqkv_sb[:, 1], wqkv_d.ap()[:, 1])

            def x_piece(j):
                nc.sync.dma_start(
                    xT_sb[:, :, j * qw:(j + 1) * qw],
                    xT_d.ap()[:, :, j * qw:(j + 1) * qw],
                )

            x_piece(0)
            nc.sync.dma_start(wqkv_sb[:, 0], wqkv_d.ap()[:, 0])
            nc.sync.dma_start(wqkv_sb[:, 2], wqkv_d.ap()[:, 2])
            for j in range(1, n_j):
                x_piece(j)
            nc.sync.dma_start(wp_sb[:], wp_d.ap())

            ident = consts.tile([P, P], bf16)
            make_identity(nc, ident[:])
            # ones row for the reciprocal partition-broadcast matmul
            ones_f32 = consts.tile([P, DH], f32)
            nc.gpsimd.memset(ones_f32[0:1, :], 1.0)
            ones_sb = consts.tile([P, DH], f32r)
            nc.vector.tensor_copy(ones_sb[0:1, :], ones_f32[0:1, :])

            # ---- persistent activations ----
            if QK_FP8:
                # pair-packed for DoubleRow scores: [64, 2, n]
                #   partition p: head = p//32, dh = (p%32) + 32*slot
                qT_sb = qkpool.tile([DH, 2, n], fp8)
                kT_sb = qkpool.tile([DH, 2, n], fp8)
            else:
                qT_sb = qkpool.tile([P, n], bf16)
                kT_sb = qkpool.tile([P, n], bf16)
            vT_sb = qkpool.tile([P, n], bf16)       # feature-major V.T (16x)
            # PV V layout: [seq-in-chunk, mc, [V_h0|1|V_h1|1]]
            v_sb = qkpool.tile([P, m_chunks, 2 * (DH + 1)], bf16)
            nc.gpsimd.memset(v_sb[:, :, DH:DH + 1], 1.0)
            nc.gpsimd.memset(v_sb[:, :, 2 * DH + 1:2 * DH + 2], 1.0)

            # ===== PSUM pools (8 banks):
            #   sps:  scores [128, 2*NB] f32 -> 2 banks x SPS_BUFS
            #   pvps: PV acc [128, 2*NB] f32 -> 2 banks, single buffered
            #   auxp: [128, 512] f32 1-bank tiles x2 (QKV acc / transposes / proj)
            sps = tc.alloc_tile_pool(name="sps", bufs=SPS_BUFS, space="PSUM")
            pvps = tc.alloc_tile_pool(name="pvps", bufs=1, space="PSUM")
            auxp = tc.alloc_tile_pool(name="auxp", bufs=2, space="PSUM")

            # ---- engine-dispatched op emitters ----
            def drain(dst, src, bias=None, act_c=612.0, dve_c=658.0, forced=None):
                """PSUM->SBUF copy (+per-partition bias) on ACT or DVE."""
                eng = pick(act_c, dve_c, forced)
                if eng == "act":
                    if bias is None:
                        nc.scalar.copy(dst, src)
                    else:
                        nc.scalar.activation(dst, src, AF.Identity, bias=bias)
                else:
                    if bias is None:
                        nc.vector.tensor_copy(dst, src)
                    else:
                        nc.vector.tensor_scalar(dst, src, bias, None, ALU.add)

            def exp_op(dst_bf16, src_ps):
                if dve_exps[0] < EXP_DVE_MAX:
                    eng = pick(612.0, 658.0)
                else:
                    eng = pick(612.0, 658.0, forced="act")
                if eng == "act":
                    nc.scalar.activation(dst_bf16, src_ps, AF.Exp, scale=SCALE)
                else:
                    dve_exps[0] += 1
                    nc.vector.tensor_scalar(
                        dst_bf16.bitcast(u16), src_ps, float(SCH_A), float(SCH_B),
                        ALU.mult, ALU.add,
                    )

            # ---- PE warmup: burn the p-state ramp while DMAs land ----
            if WARMUP_MM:
                wps = auxp.tile([P, P], bf16, tag="aux", name="warm")
                for i in range(WARMUP_MM):
                    nc.tensor.transpose(wps[:], ident[:], ident[:])

            # ================= QKV ===========================================
            def qkv_j(j, parts):
                jsl = slice(j * qw, (j + 1) * qw)
                for part in parts:
                    ps = auxp.tile([P, qw], f32, tag="aux", name=f"qkv_{part}_{j}")
                    if part == 2:
                        for o in range(D_CHUNKS):
                            nc.tensor.matmul(
                                ps[:], wqkv_sb[:, part, o], xT_sb[:, o, jsl],
                                start=(o == 0), stop=(o == D_CHUNKS - 1),
                            )
                        drain(vT_sb[:, jsl], ps[:],
                              bias=bqkv_sb[:, 2:3])
                        # V.T -> v_sb via PE transpose per m-chunk
                        for mc in range(j * qw // P, (j + 1) * qw // P):
                            tp = auxp.tile([P, P], bf16, tag="aux", name=f"tp_{mc}")
                            nc.tensor.transpose(
                                tp[:], vT_sb[:, mc * P:(mc + 1) * P], ident[:]
                            )
                            # both heads in one strided copy [128, 2, 64]
                            drain(
                                v_sb[:, mc, :].rearrange(
                                    "p (h c) -> p h c", h=2
                                )[:, :, 0:DH],
                                tp[:].rearrange("p (h c) -> p h c", h=2),
                                act_c=292.0, dve_c=258.0,
                            )
                    else:
                        for o in range(D_CHUNKS):
                            nc.tensor.matmul(
                                ps[:], wqkv_sb[:, part, o], xT_sb[:, o, jsl],
                                start=(o == 0), stop=(o == D_CHUNKS - 1),
                            )
                        dst = qT_sb if part == 0 else kT_sb
                        if QK_FP8:
                            # pair-split drains into [64, 2, n] layout
                            for s in range(2):
                                drain(
                                    dst[:, s, jsl],
                                    ps[s * DH:(s + 1) * DH, :],
                                    bias=bqkv_sb[s * DH:(s + 1) * DH, part:part + 1],
                                )
                        else:
                            drain(dst[:, jsl], ps[:], bias=bqkv_sb[:, part:part + 1])

            # ================= attention =====================================
            with (
                tc.tile_pool(name="espool", bufs=ES_BUFS) as espool,
                tc.tile_pool(name="opool", bufs=2) as opool,
                tc.tile_pool(name="outpool", bufs=2) as outpool,
                tc.tile_pool(name="rpool", bufs=2) as rpool,
            ):
                def attn_sc(b, row0, nbb, mcs):
                    nsl = slice(row0, row0 + nbb)
                    out = []
                    for mc in mcs:
                        pair = []
                        for h in range(HPC):
                            s_ps = sps.tile([P, nbb], f32, tag="s",
                                            name=f"s_ps_{b}_{mc}_{h}")
                            if QK_FP8:
                                nc.tensor.matmul(
                                    s_ps[:],
                                    kT_sb[32 * h:32 * (h + 1), :,
                                          mc * P:(mc + 1) * P],
                                    qT_sb[32 * h:32 * (h + 1), :, nsl],
                                    perf_mode=DR,
                                )
                            else:
                                nc.tensor.matmul(
                                    s_ps[:],
                                    kT_sb[h * DH:(h + 1) * DH,
                                          mc * P:(mc + 1) * P],
                                    qT_sb[h * DH:(h + 1) * DH, nsl],
                                    tile_position=(h * DH, 0),
                                )
                            es = espool.tile([P, nbb], bf16, tag="es",
                                             name=f"es_{b}_{mc}_{h}")
                            exp_op(es[:], s_ps[:])
                            pair.append(es)
                        out.append((mc, pair))
                    return out

                def attn_pv(nbb, pvs, mc_es):
                    for mc, pair in mc_es:
                        for h in range(HPC):
                            nc.tensor.matmul(
                                pvs[0:DH + 1, h * nbb:(h + 1) * nbb],
                                v_sb[:, mc, h * (DH + 1):(h + 1) * (DH + 1)],
                                pair[h][:],
                                start=(mc == 0),
                                stop=(mc == m_chunks - 1),
                            )

                def attn_mc_group(b, row0, nbb, pvs, mcs):
                    attn_pv(nbb, pvs, attn_sc(b, row0, nbb, mcs))

                def division(b, nbb, pvs):
                    # O.T = O'.T / rowsum, heads stacked on partitions.
                    rt = rpool.tile([P, HPC * nbb], f32r, tag="recip",
                                    name=f"rt_{b}")
                    rb = rpool.tile([DH, HPC * nbb], f32, tag="rbcast",
                                    name=f"rb_{b}")
                    oT = opool.tile([P, nbb], bf16, tag="oT", name=f"oT_{b}")
                    with nc.allow_low_precision(reason="f32r recip"):
                        nc.vector.reciprocal(rt[0:1, :], pvs[DH:DH + 1, :])
                    load["dve"] += 1190.0
                    for h in range(HPC):
                        hs = slice(h * nbb, (h + 1) * nbb)
                        rb_ps = auxp.tile([P, nbb], f32, tag="aux",
                                          name=f"rb_{b}_{h}")
                        nc.tensor.matmul(rb_ps[0:DH, :], ones_sb[0:1, :],
                                         rt[0:1, hs])
                        drain(rb[:, hs], rb_ps[0:DH, :])
                        nc.vector.tensor_mul(
                            oT[h * DH:(h + 1) * DH, :],
                            pvs[0:DH, hs],
                            rb[:, hs],
                        )
                        load["dve"] += 658.0
                    return oT

                def projection(b, row0, nbb, oT, last=False):
                    nch = nbb // P
                    out_sb = outpool.tile([P, nch, D], f16, tag="out",
                                          name=f"out_{b}")
                    for j in range(nch):
                        for half in range(D // 512):
                            pp = auxp.tile([P, 512], f32, tag="aux",
                                           name=f"pp_{b}_{j}_{half}")
                            nc.tensor.matmul(
                                pp[:],
                                oT[:, j * P:(j + 1) * P],
                                wp_sb[:, half * 512:(half + 1) * 512],
                            )
                            drain(out_sb[:, j, half * 512:(half + 1) * 512], pp[:])
                    nc.sync.dma_start(
                        out_d.ap()[row0:row0 + nbb, :].rearrange(
                            "(c p) d -> p c d", p=P
                        ),
                        out_sb[:],
                    )

                blocks = [nb] * n_blocks
                pending = None   # (b, row0, nbb, oT) awaiting projection
                row0 = 0
                for b, nbb in enumerate(blocks):
                    pvs = pvps.tile([P, HPC * nbb], f32, tag="pv", name=f"pv_{b}")
                    if b == 0:
                        # fine interleave with QKV j-sweeps
                        mcs_per_j = qw // P
                        for j in range(n_j):
                            qkv_j(j, parts=(1, 0) if j == 0 else (1,))
                            mc_es = attn_sc(b, row0, nbb,
                                            range(j * mcs_per_j,
                                                  (j + 1) * mcs_per_j))
                            qkv_j(j, parts=(2,))
                            attn_pv(nbb, pvs, mc_es)
                        if n_j > 1:
                            qkv_j(1, parts=(0,))
                    else:
                        split = min(4, m_chunks)
                        attn_mc_group(b, row0, nbb, pvs, range(0, split))
                        if pending is not None:
                            projection(*pending)
                            pending = None
                        if b + 1 < n_j:
                            qkv_j(b + 1, parts=(0,))
                        attn_mc_group(b, row0, nbb, pvs, range(split, m_chunks))
                    oT = division(b, nbb, pvs)
                    if pending is not None:
                        projection(*pending)
                        pending = None
                    pending = (b, row0, nbb, oT)
                    row0 += nbb
                projection(*pending, last=True)

            auxp.release()
            pvps.release()
            sps.release()

    nc.compile()
    return nc


def _host_prep(x, w_qkv, b_qkv, w_proj, n=N):
    """Per-core input maps (dtypes match the DRAM tensor declarations)."""
    import ml_dtypes

    fp8 = ml_dtypes.float8_e4m3
    bf = ml_dtypes.bfloat16

    xT = np.ascontiguousarray(
        x.T.reshape(D_CHUNKS, P, n).transpose(1, 0, 2).astype(bf)
    )

    if QK_FP8:
        # q/k feature order for PSUM partition j:
        #   slot s = j // 64, head h = (j % 64) // 32, dh = (j % 32) + 32*s
        j_idx = np.arange(P)
        s_idx, jj = j_idx // DH, j_idx % DH
        qk_feat = (jj // 32) * DH + (jj % 32) + 32 * s_idx
    else:
        qk_feat = np.arange(P)

    in_maps = []
    for c in range(NCORES):
        parts = []
        border = []
        for part in range(3):
            rows = w_qkv[part * D + c * P:part * D + (c + 1) * P, :]
            feat = qk_feat if part != 2 else np.arange(P)
            wperm = rows[feat, :]                          # [128 feat, D]
            # lhsT layout [p, o, feat]
            parts.append(wperm.T.reshape(D_CHUNKS, P, P).transpose(1, 0, 2))
            border.append(b_qkv[part * D + c * P:part * D + (c + 1) * P][feat])
        wqkv = np.ascontiguousarray(np.stack(parts, axis=1).astype(bf))
        bqkv = np.ascontiguousarray(np.stack(border, axis=1).astype(np.float32))
        wpT = np.ascontiguousarray(w_proj[:, c * P:(c + 1) * P].T.astype(bf))
        in_maps.append({"xT": xT, "wqkv": wqkv, "wpT": wpT, "bqkv": bqkv})
    return in_maps


_NC_CACHE = {}


def run(x, w_qkv, b_qkv, w_proj, b_proj, trace=False, n=N, nb=None, **spmd_kwargs):
    from concourse.bass_utils import run_bass_kernel_spmd

    if nb is None:
        nb = NB
    key = (n, nb, SPS_BUFS, ES_BUFS, QK_FP8, EXP_DVE_MAX)
    if key not in _NC_CACHE:
        _NC_CACHE[key] = _build_nc(n=n, nb=nb)
    nc = _NC_CACHE[key]

    in_maps = _host_prep(
        np.asarray(x), np.asarray(w_qkv), np.asarray(b_qkv), np.asarray(w_proj), n=n
    )
    results = run_bass_kernel_spmd(
        nc, in_maps, core_ids=list(range(NCORES)), trace=trace, **spmd_kwargs
    )
    acc = np.zeros((n, D), dtype=np.float64)
    for c in range(NCORES):
        acc += results.results[c]["out_part"].astype(np.float64)
    acc += np.asarray(b_proj).astype(np.float64)
    return acc.astype(np.float32), results


def kernel(x, w_qkv, b_qkv, w_proj, b_proj):
    out, _ = run(x, w_qkv, b_qkv, w_proj, b_proj, trace=False)
    return out


# revision 17
# speedup vs baseline: 1.0439x; 1.0287x over previous
"""Multi-head attention (N=2048, D=1024, H=16) on 8 TRN2 NeuronCores.

Sharding: tensor-parallel over heads (2 heads / core). x is replicated,
each core computes QKV / scores / softmax / PV / out-proj for its 2
heads, producing a partial (N, D) projection output in fp16. The
all-reduce over cores is the host-side f64 sum of the 8 partials
(+ b_proj), cast back to f32.

Precision/speed mix (softmax-weight noise passes through ~1:1, so the
logits path stays bf16-accurate; only noise that averages out rides fp8):
  QKV(q,k) bf16 matmuls (exact logits);  QKV(v) fp8e4m3 DoubleRow with
        16x host-scaled weights (v noise averages out in PV; the 1/16
        is folded into w_proj).
  q/k   optionally stored fp8 pair-packed [64, 2, n] so scores can run
        DoubleRow with Ki=32 (QK_FP8 knob; ~1.3% logit noise).
  V.T   --PE transpose (bf16)--> v_sb [seq, mc, 2*(DH+1)] with ones
        columns for the PV rowsum.
  exp   bf16 es, split between ACT (true exp) and DVE (Schraudolph:
        one tensor_scalar into uint16 = bf16 bit pattern). A greedy
        balancer assigns every exp/drain op to the less-loaded engine;
        EXP_DVE_MAX caps how many exps may take the (noisier) DVE path.
  PV    bf16 with rowsum ones-column.
  div   DVE reciprocal + DMA round-trip partition-broadcast via DRAM
        + DVE muls.
  proj  bf16 matmuls; f16 drains; one DMA per 512-row block.
"""

import os
import sys

import numpy as np

for _p in ("/opt/trn_rl_repo",):
    if os.path.isdir(_p) and _p not in sys.path:
        sys.path.insert(0, _p)

N, D, H = 2048, 1024, 16
DH = D // H                 # 64
NCORES = 8
HPC = H // NCORES           # 2 heads per core
P = 128
SCALE = 1.0 / DH ** 0.5

D_CHUNKS = D // P           # 8
G_CHUNKS = D_CHUNKS // 2    # 4 d-chunk pairs (DoubleRow v)
WSCALE = 16.0               # host-side fp8 v-weight scale (undone in w_proj)

NB = int(os.environ.get("ATTN_NB", "512"))                 # query-block size
SPS_BUFS = int(os.environ.get("ATTN_SPS_BUFS", "4"))
ES_BUFS = int(os.environ.get("ATTN_ES_BUFS", "10"))
QK_FP8 = os.environ.get("ATTN_QK_FP8", "0") == "1"         # DoubleRow scores
EXP_DVE_MAX = int(os.environ.get("ATTN_EXP_DVE_MAX", "64"))
WARMUP_MM = int(os.environ.get("ATTN_WARMUP", "26"))

# exp(s * SCALE) from psum scores; bf16 Schraudolph constants
SCH_A = 128.0 * np.log2(np.e) * SCALE                      # uint16 mult
SCH_B = float(os.environ.get("ATTN_SCH_B", "16249.1"))     # uint16 bias


def _build_nc(n=N, nb=NB):
    """Build the per-core Bass module (SPMD: identical program, per-core data)."""
    import concourse.bass as bass  # noqa: F401
    import concourse.mybir as mybir
    import concourse.tile as tile
    from concourse import bacc
    from concourse.masks import make_identity

    f32 = mybir.dt.float32
    f32r = mybir.dt.float32r
    bf16 = mybir.dt.bfloat16
    f16 = mybir.dt.float16
    fp8 = mybir.dt.float8e4
    u16 = mybir.dt.uint16
    AF = mybir.ActivationFunctionType
    DR = mybir.MatmulPerfMode.DoubleRow
    ALU = mybir.AluOpType

    m_chunks = n // P
    n_blocks = n // nb
    qk_dt = fp8 if QK_FP8 else bf16

    nc = bacc.Bacc(
        "TRN2",
        target_bir_lowering=False,
        debug=False,
        enable_asserts=True,
        num_devices=NCORES,
    )

    xT_d = nc.dram_tensor("xT", (P, D_CHUNKS, n), bf16, kind="ExternalInput")
    wqkv_d = nc.dram_tensor("wqkv", (P, 3, D_CHUNKS, P), bf16, kind="ExternalInput")
    wp_d = nc.dram_tensor("wpT", (P, D), bf16, kind="ExternalInput")
    bqkv_d = nc.dram_tensor("bqkv", (P, 3), f32, kind="ExternalInput")
    out_d = nc.dram_tensor("out_part", (n, D), f16, kind="ExternalOutput")

    # ---- static ACT/DVE load balancer ----
    load = {"act": 0.0, "dve": 0.0}
    dve_exps = [0]

    def pick(act_cost, dve_cost, forced=None):
        if forced is None:
            eng = "act" if load["act"] + act_cost <= load["dve"] + dve_cost else "dve"
        else:
            eng = forced
        load[eng] += act_cost if eng == "act" else dve_cost
        return eng

    with tile.TileContext(nc) as tc:
        with (
            tc.tile_pool(name="consts", bufs=1) as consts,
            tc.tile_pool(name="xpool", bufs=1) as xpool,
            tc.tile_pool(name="qkpool", bufs=1) as qkpool,
        ):
            # ---- inputs ----
            wqkv_sb = consts.tile([P, 3, D_CHUNKS, P], bf16)
            wp_sb = consts.tile([P, D], bf16)
            bqkv_sb = consts.tile([P, 3], f32)
            xT_sb = xpool.tile([P, D_CHUNKS, n], bf16)

            qw = min(512, n)
            n_j = n // qw

            # identity/ones first so PE warmup can start immediately
            ident = consts.tile([P, P], bf16)
            make_identity(nc, ident[:])
            # ones row for the reciprocal partition-broadcast matmul
            ones_f32 = consts.tile([P, DH], f32)
            nc.gpsimd.memset(ones_f32[0:1, :], 1.0)
            ones_sb = consts.tile([P, DH], f32r)
            nc.vector.tensor_copy(ones_sb[0:1, :], ones_f32[0:1, :])

            nc.sync.dma_start(bqkv_sb[:], bqkv_d.ap())
            # k first: it gates the first scores matmul
            nc.sync.dma_start(wqkv_sb[:, 1], wqkv_d.ap()[:, 1])

            def x_piece(j):
                nc.sync.dma_start(
                    xT_sb[:, :, j * qw:(j + 1) * qw],
                    xT_d.ap()[:, :, j * qw:(j + 1) * qw],
                )

            # j0 in halves so the first QKV matmuls start sooner
            nc.sync.dma_start(xT_sb[:, 0:4, 0:qw], xT_d.ap()[:, 0:4, 0:qw])
            nc.sync.dma_start(wqkv_sb[:, 0], wqkv_d.ap()[:, 0])
            nc.sync.dma_start(xT_sb[:, 4:8, 0:qw], xT_d.ap()[:, 4:8, 0:qw])
            nc.sync.dma_start(wqkv_sb[:, 2], wqkv_d.ap()[:, 2])
            for j in range(1, n_j):
                x_piece(j)
            nc.sync.dma_start(wp_sb[:], wp_d.ap())

            # ---- persistent activations ----
            if QK_FP8:
                # pair-packed for DoubleRow scores: [64, 2, n]
                #   partition p: head = p//32, dh = (p%32) + 32*slot
                qT_sb = qkpool.tile([DH, 2, n], fp8)
                kT_sb = qkpool.tile([DH, 2, n], fp8)
            else:
                qT_sb = qkpool.tile([P, n], bf16)
                kT_sb = qkpool.tile([P, n], bf16)
            vT_sb = qkpool.tile([P, n], bf16)       # feature-major V.T (16x)
            # PV V layout: [seq-in-chunk, mc, [V_h0|1|V_h1|1]]
            v_sb = qkpool.tile([P, m_chunks, 2 * (DH + 1)], bf16)
            nc.gpsimd.memset(v_sb[:, :, DH:DH + 1], 1.0)
            nc.gpsimd.memset(v_sb[:, :, 2 * DH + 1:2 * DH + 2], 1.0)

            # ===== PSUM pools (8 banks):
            #   sps:  scores [128, 2*NB] f32 -> 2 banks x SPS_BUFS
            #   pvps: PV acc [128, 2*NB] f32 -> 2 banks, single buffered
            #   auxp: [128, 512] f32 1-bank tiles x2 (QKV acc / transposes / proj)
            sps = tc.alloc_tile_pool(name="sps", bufs=SPS_BUFS, space="PSUM")
            pvps = tc.alloc_tile_pool(name="pvps", bufs=1, space="PSUM")
            auxp = tc.alloc_tile_pool(name="auxp", bufs=2, space="PSUM")

            # ---- engine-dispatched op emitters ----
            def drain(dst, src, bias=None, act_c=612.0, dve_c=658.0, forced=None):
                """PSUM->SBUF copy (+per-partition bias) on ACT or DVE."""
                eng = pick(act_c, dve_c, forced)
                if eng == "act":
                    if bias is None:
                        nc.scalar.copy(dst, src)
                    else:
                        nc.scalar.activation(dst, src, AF.Identity, bias=bias)
                else:
                    if bias is None:
                        nc.vector.tensor_copy(dst, src)
                    else:
                        nc.vector.tensor_scalar(dst, src, bias, None, ALU.add)

            def exp_op(dst_bf16, src_ps):
                if dve_exps[0] < EXP_DVE_MAX:
                    eng = pick(612.0, 658.0)
                else:
                    eng = pick(612.0, 658.0, forced="act")
                if eng == "act":
                    nc.scalar.activation(dst_bf16, src_ps, AF.Exp, scale=SCALE)
                else:
                    dve_exps[0] += 1
                    nc.vector.tensor_scalar(
                        dst_bf16.bitcast(u16), src_ps, float(SCH_A), float(SCH_B),
                        ALU.mult, ALU.add,
                    )

            # ---- PE warmup: burn the p-state ramp while DMAs land ----
            if WARMUP_MM:
                wps = auxp.tile([P, P], bf16, tag="aux", name="warm")
                for i in range(WARMUP_MM):
                    nc.tensor.transpose(wps[:], ident[:], ident[:])

            # ================= QKV ===========================================
            def qkv_j(j, parts):
                jsl = slice(j * qw, (j + 1) * qw)
                for part in parts:
                    ps = auxp.tile([P, qw], f32, tag="aux", name=f"qkv_{part}_{j}")
                    if part == 2:
                        for o in range(D_CHUNKS):
                            nc.tensor.matmul(
                                ps[:], wqkv_sb[:, part, o], xT_sb[:, o, jsl],
                                start=(o == 0), stop=(o == D_CHUNKS - 1),
                            )
                        drain(vT_sb[:, jsl], ps[:],
                              bias=bqkv_sb[:, 2:3])
                        # V.T -> v_sb via PE transpose per m-chunk
                        for mc in range(j * qw // P, (j + 1) * qw // P):
                            tp = auxp.tile([P, P], bf16, tag="aux", name=f"tp_{mc}")
                            nc.tensor.transpose(
                                tp[:], vT_sb[:, mc * P:(mc + 1) * P], ident[:]
                            )
                            # both heads in one strided copy [128, 2, 64]
                            drain(
                                v_sb[:, mc, :].rearrange(
                                    "p (h c) -> p h c", h=2
                                )[:, :, 0:DH],
                                tp[:].rearrange("p (h c) -> p h c", h=2),
                                act_c=292.0, dve_c=258.0,
                            )
                    else:
                        for o in range(D_CHUNKS):
                            nc.tensor.matmul(
                                ps[:], wqkv_sb[:, part, o], xT_sb[:, o, jsl],
                                start=(o == 0), stop=(o == D_CHUNKS - 1),
                            )
                        dst = qT_sb if part == 0 else kT_sb
                        if QK_FP8:
                            # pair-split drains into [64, 2, n] layout
                            for s in range(2):
                                drain(
                                    dst[:, s, jsl],
                                    ps[s * DH:(s + 1) * DH, :],
                                    bias=bqkv_sb[s * DH:(s + 1) * DH, part:part + 1],
                                )
                        else:
                            drain(dst[:, jsl], ps[:], bias=bqkv_sb[:, part:part + 1])

            # ================= attention =====================================
            with (
                tc.tile_pool(name="espool", bufs=ES_BUFS) as espool,
                tc.tile_pool(name="opool", bufs=2) as opool,
                tc.tile_pool(name="outpool", bufs=2) as outpool,
                tc.tile_pool(name="rpool", bufs=2) as rpool,
            ):
                def attn_sc(b, row0, nbb, mcs):
                    nsl = slice(row0, row0 + nbb)
                    out = []
                    for mc in mcs:
                        pair = []
                        for h in range(HPC):
                            s_ps = sps.tile([P, nbb], f32, tag="s",
                                            name=f"s_ps_{b}_{mc}_{h}")
                            if QK_FP8:
                                nc.tensor.matmul(
                                    s_ps[:],
                                    kT_sb[32 * h:32 * (h + 1), :,
                                          mc * P:(mc + 1) * P],
                                    qT_sb[32 * h:32 * (h + 1), :, nsl],
                                    perf_mode=DR,
                                )
                            else:
                                nc.tensor.matmul(
                                    s_ps[:],
                                    kT_sb[h * DH:(h + 1) * DH,
                                          mc * P:(mc + 1) * P],
                                    qT_sb[h * DH:(h + 1) * DH, nsl],
                                    tile_position=(h * DH, 0),
                                )
                            es = espool.tile([P, nbb], bf16, tag="es",
                                             name=f"es_{b}_{mc}_{h}")
                            exp_op(es[:], s_ps[:])
                            pair.append(es)
                        out.append((mc, pair))
                    return out

                def attn_pv(nbb, pvs, mc_es):
                    for mc, pair in mc_es:
                        for h in range(HPC):
                            nc.tensor.matmul(
                                pvs[0:DH + 1, h * nbb:(h + 1) * nbb],
                                v_sb[:, mc, h * (DH + 1):(h + 1) * (DH + 1)],
                                pair[h][:],
                                start=(mc == 0),
                                stop=(mc == m_chunks - 1),
                            )

                def attn_mc_group(b, row0, nbb, pvs, mcs):
                    attn_pv(nbb, pvs, attn_sc(b, row0, nbb, mcs))

                def division(b, nbb, pvs):
                    # O.T = O'.T / rowsum, heads stacked on partitions.
                    rt = rpool.tile([P, HPC * nbb], f32r, tag="recip",
                                    name=f"rt_{b}")
                    rb = rpool.tile([DH, HPC * nbb], f32, tag="rbcast",
                                    name=f"rb_{b}")
                    oT = opool.tile([P, nbb], bf16, tag="oT", name=f"oT_{b}")
                    with nc.allow_low_precision(reason="f32r recip"):
                        nc.vector.reciprocal(rt[0:1, :], pvs[DH:DH + 1, :])
                    load["dve"] += 1190.0
                    for h in range(HPC):
                        hs = slice(h * nbb, (h + 1) * nbb)
                        rb_ps = auxp.tile([P, nbb], f32, tag="aux",
                                          name=f"rb_{b}_{h}")
                        nc.tensor.matmul(rb_ps[0:DH, :], ones_sb[0:1, :],
                                         rt[0:1, hs])
                        drain(rb[:, hs], rb_ps[0:DH, :])
                        nc.vector.tensor_mul(
                            oT[h * DH:(h + 1) * DH, :],
                            pvs[0:DH, hs],
                            rb[:, hs],
                        )
                        load["dve"] += 658.0
                    return oT

                def proj_piece(b, row0, oT, out_sb, j, dma_now=False):
                    for half in range(D // 512):
                        pp = auxp.tile([P, 512], f32, tag="aux",
                                       name=f"pp_{b}_{j}_{half}")
                        nc.tensor.matmul(
                            pp[:],
                            oT[:, j * P:(j + 1) * P],
                            wp_sb[:, half * 512:(half + 1) * 512],
                        )
                        drain(out_sb[:, j, half * 512:(half + 1) * 512], pp[:])
                    if dma_now:
                        nc.sync.dma_start(
                            out_d.ap()[row0 + j * P:row0 + (j + 1) * P, :],
                            out_sb[:, j],
                        )

                def projection(b, row0, nbb, oT, last=False):
                    nch = nbb // P
                    out_sb = outpool.tile([P, nch, D], f16, tag="out",
                                          name=f"out_{b}")
                    for j in range(nch):
                        proj_piece(b, row0, oT, out_sb, j, dma_now=last)
                    if not last:
                        nc.sync.dma_start(
                            out_d.ap()[row0:row0 + nbb, :].rearrange(
                                "(c p) d -> p c d", p=P
                            ),
                            out_sb[:],
                        )

                blocks = [nb] * n_blocks
                pending = None   # (b, row0, nbb, oT, out_sb) awaiting projection
                row0 = 0
                for b, nbb in enumerate(blocks):
                    pvs = pvps.tile([P, HPC * nbb], f32, tag="pv", name=f"pv_{b}")
                    if b == 0:
                        # fine interleave with QKV j-sweeps
                        mcs_per_j = qw // P
                        for j in range(n_j):
                            qkv_j(j, parts=(1, 0) if j == 0 else (1,))
                            mc_es = attn_sc(b, row0, nbb,
                                            range(j * mcs_per_j,
                                                  (j + 1) * mcs_per_j))
                            qkv_j(j, parts=(2,))
                            attn_pv(nbb, pvs, mc_es)
                        if n_j > 1:
                            qkv_j(1, parts=(0,))
                    else:
                        # pending projection pieces spread between mc groups
                        pj = [None]

                        def proj_next():
                            if pending is None:
                                return
                            pb, prow0, pnbb, poT, pout = pending
                            if pj[0] is None:
                                pj[0] = 0
                            if pj[0] < pnbb // P:
                                proj_piece(pb, prow0, poT, pout, pj[0])
                                pj[0] += 1

                        attn_mc_group(b, row0, nbb, pvs, range(0, 4))
                        proj_next()
                        if b + 1 < n_j:
                            qkv_j(b + 1, parts=(0,))
                        attn_mc_group(b, row0, nbb, pvs, range(4, 8))
                        proj_next()
                        attn_mc_group(b, row0, nbb, pvs, range(8, 12))
                        proj_next()
                        attn_mc_group(b, row0, nbb, pvs, range(12, m_chunks))
                        proj_next()
                        if pending is not None:
                            pb, prow0, pnbb, poT, pout = pending
                            nc.sync.dma_start(
                                out_d.ap()[prow0:prow0 + pnbb, :].rearrange(
                                    "(c p) d -> p c d", p=P
                                ),
                                pout[:],
                            )
                            pending = None
                    oT = division(b, nbb, pvs)
                    out_sb = outpool.tile([P, nbb // P, D], f16, tag="out",
                                          name=f"out_{b}")
                    pending = (b, row0, nbb, oT, out_sb)
                    row0 += nbb
                # final block: pipelined per-j projection + per-j DMA
                pb, prow0, pnbb, poT, pout = pending
                for j in range(pnbb // P):
                    proj_piece(pb, prow0, poT, pout, j, dma_now=True)

            auxp.release()
            pvps.release()
            sps.release()

    nc.compile()
    return nc


def _host_prep(x, w_qkv, b_qkv, w_proj, n=N):
    """Per-core input maps (dtypes match the DRAM tensor declarations)."""
    import ml_dtypes

    fp8 = ml_dtypes.float8_e4m3
    bf = ml_dtypes.bfloat16

    xT = np.ascontiguousarray(
        x.T.reshape(D_CHUNKS, P, n).transpose(1, 0, 2).astype(bf)
    )

    if QK_FP8:
        # q/k feature order for PSUM partition j:
        #   slot s = j // 64, head h = (j % 64) // 32, dh = (j % 32) + 32*s
        j_idx = np.arange(P)
        s_idx, jj = j_idx // DH, j_idx % DH
        qk_feat = (jj // 32) * DH + (jj % 32) + 32 * s_idx
    else:
        qk_feat = np.arange(P)

    in_maps = []
    for c in range(NCORES):
        parts = []
        border = []
        for part in range(3):
            rows = w_qkv[part * D + c * P:part * D + (c + 1) * P, :]
            feat = qk_feat if part != 2 else np.arange(P)
            wperm = rows[feat, :]                          # [128 feat, D]
            # lhsT layout [p, o, feat]
            parts.append(wperm.T.reshape(D_CHUNKS, P, P).transpose(1, 0, 2))
            border.append(b_qkv[part * D + c * P:part * D + (c + 1) * P][feat])
        wqkv = np.ascontiguousarray(np.stack(parts, axis=1).astype(bf))
        bqkv = np.ascontiguousarray(np.stack(border, axis=1).astype(np.float32))
        wpT = np.ascontiguousarray(w_proj[:, c * P:(c + 1) * P].T.astype(bf))
        in_maps.append({"xT": xT, "wqkv": wqkv, "wpT": wpT, "bqkv": bqkv})
    return in_maps


_NC_CACHE = {}


def run(x, w_qkv, b_qkv, w_proj, b_proj, trace=False, n=N, nb=None, **spmd_kwargs):
    from concourse.bass_utils import run_bass_kernel_spmd

    if nb is None:
        nb = NB
    key = (n, nb, SPS_BUFS, ES_BUFS, QK_FP8, EXP_DVE_MAX)
    if key not in _NC_CACHE:
        _NC_CACHE[key] = _build_nc(n=n, nb=nb)
    nc = _NC_CACHE[key]

    in_maps = _host_prep(
        np.asarray(x), np.asarray(w_qkv), np.asarray(b_qkv), np.asarray(w_proj), n=n
    )
    results = run_bass_kernel_spmd(
        nc, in_maps, core_ids=list(range(NCORES)), trace=trace, **spmd_kwargs
    )
    acc = np.zeros((n, D), dtype=np.float64)
    for c in range(NCORES):
        acc += results.results[c]["out_part"].astype(np.float64)
    acc += np.asarray(b_proj).astype(np.float64)
    return acc.astype(np.float32), results


def kernel(x, w_qkv, b_qkv, w_proj, b_proj):
    out, _ = run(x, w_qkv, b_qkv, w_proj, b_proj, trace=False)
    return out


# revision 18
# speedup vs baseline: 1.0686x; 1.0236x over previous
"""Multi-head attention (N=2048, D=1024, H=16) on 8 TRN2 NeuronCores.

Sharding: tensor-parallel over heads (2 heads / core). x is replicated,
each core computes QKV / scores / softmax / PV / out-proj for its 2
heads, producing a partial (N, D) projection output in fp16. The
all-reduce over cores is the host-side f64 sum of the 8 partials
(+ b_proj), cast back to f32.

Precision/speed mix (softmax-weight noise passes through ~1:1, so the
logits path stays bf16-accurate; only noise that averages out rides fp8):
  QKV(q,k) bf16 matmuls (exact logits);  QKV(v) fp8e4m3 DoubleRow with
        16x host-scaled weights (v noise averages out in PV; the 1/16
        is folded into w_proj).
  q/k   optionally stored fp8 pair-packed [64, 2, n] so scores can run
        DoubleRow with Ki=32 (QK_FP8 knob; ~1.3% logit noise).
  V.T   --PE transpose (bf16)--> v_sb [seq, mc, 2*(DH+1)] with ones
        columns for the PV rowsum.
  exp   bf16 es, split between ACT (true exp) and DVE (Schraudolph:
        one tensor_scalar into uint16 = bf16 bit pattern). A greedy
        balancer assigns every exp/drain op to the less-loaded engine;
        EXP_DVE_MAX caps how many exps may take the (noisier) DVE path.
  PV    bf16 with rowsum ones-column.
  div   DVE reciprocal + DMA round-trip partition-broadcast via DRAM
        + DVE muls.
  proj  bf16 matmuls; f16 drains; one DMA per 512-row block.
"""

import os
import sys

import numpy as np

for _p in ("/opt/trn_rl_repo",):
    if os.path.isdir(_p) and _p not in sys.path:
        sys.path.insert(0, _p)

N, D, H = 2048, 1024, 16
DH = D // H                 # 64
NCORES = 8
HPC = H // NCORES           # 2 heads per core
P = 128
SCALE = 1.0 / DH ** 0.5

D_CHUNKS = D // P           # 8
G_CHUNKS = D_CHUNKS // 2    # 4 d-chunk pairs (DoubleRow v)
WSCALE = 16.0               # host-side fp8 v-weight scale (undone in w_proj)

NB = int(os.environ.get("ATTN_NB", "512"))                 # query-block size
SPS_BUFS = int(os.environ.get("ATTN_SPS_BUFS", "4"))
ES_BUFS = int(os.environ.get("ATTN_ES_BUFS", "10"))
QK_FP8 = os.environ.get("ATTN_QK_FP8", "0") == "1"         # DoubleRow scores
EXP_DVE_MAX = int(os.environ.get("ATTN_EXP_DVE_MAX", "64"))
WARMUP_MM = int(os.environ.get("ATTN_WARMUP", "36"))

# exp(s * SCALE) from psum scores; bf16 Schraudolph constants
SCH_A = 128.0 * np.log2(np.e) * SCALE                      # uint16 mult
SCH_B = float(os.environ.get("ATTN_SCH_B", "16249.1"))     # uint16 bias


def _build_nc(n=N, nb=NB):
    """Build the per-core Bass module (SPMD: identical program, per-core data)."""
    import concourse.bass as bass  # noqa: F401
    import concourse.mybir as mybir
    import concourse.tile as tile
    from concourse import bacc
    from concourse.masks import make_identity

    f32 = mybir.dt.float32
    f32r = mybir.dt.float32r
    bf16 = mybir.dt.bfloat16
    f16 = mybir.dt.float16
    fp8 = mybir.dt.float8e4
    u16 = mybir.dt.uint16
    AF = mybir.ActivationFunctionType
    DR = mybir.MatmulPerfMode.DoubleRow
    ALU = mybir.AluOpType

    m_chunks = n // P
    n_blocks = n // nb
    qk_dt = fp8 if QK_FP8 else bf16

    nc = bacc.Bacc(
        "TRN2",
        target_bir_lowering=False,
        debug=False,
        enable_asserts=True,
        num_devices=NCORES,
    )

    xT_d = nc.dram_tensor("xT", (P, D_CHUNKS, n), bf16, kind="ExternalInput")
    wqkv_d = nc.dram_tensor("wqkv", (P, 3, D_CHUNKS, P), bf16, kind="ExternalInput")
    wp_d = nc.dram_tensor("wpT", (P, D), bf16, kind="ExternalInput")
    bqkv_d = nc.dram_tensor("bqkv", (P, 3), f32, kind="ExternalInput")
    out_d = nc.dram_tensor("out_part", (n, D), f16, kind="ExternalOutput")

    # ---- static ACT/DVE load balancer ----
    load = {"act": 0.0, "dve": 0.0}
    dve_exps = [0]

    def pick(act_cost, dve_cost, forced=None):
        if forced is None:
            eng = "act" if load["act"] + act_cost <= load["dve"] + dve_cost else "dve"
        else:
            eng = forced
        load[eng] += act_cost if eng == "act" else dve_cost
        return eng

    with tile.TileContext(nc) as tc:
        with (
            tc.tile_pool(name="consts", bufs=1) as consts,
            tc.tile_pool(name="xpool", bufs=1) as xpool,
            tc.tile_pool(name="qkpool", bufs=1) as qkpool,
        ):
            # ---- inputs ----
            wqkv_sb = consts.tile([P, 3, D_CHUNKS, P], bf16)
            wp_sb = consts.tile([P, D], bf16)
            bqkv_sb = consts.tile([P, 3], f32)
            xT_sb = xpool.tile([P, D_CHUNKS, n], bf16)

            qw = min(512, n)
            n_j = n // qw

            # identity/ones first so PE warmup can start immediately
            ident = consts.tile([P, P], bf16)
            make_identity(nc, ident[:])
            # ones row for the reciprocal partition-broadcast matmul
            ones_f32 = consts.tile([P, DH], f32)
            nc.gpsimd.memset(ones_f32[0:1, :], 1.0)
            ones_sb = consts.tile([P, DH], f32r)
            nc.vector.tensor_copy(ones_sb[0:1, :], ones_f32[0:1, :])

            nc.sync.dma_start(bqkv_sb[:], bqkv_d.ap())
            # k first: it gates the first scores matmul
            nc.sync.dma_start(wqkv_sb[:, 1], wqkv_d.ap()[:, 1])

            def x_piece(j):
                nc.sync.dma_start(
                    xT_sb[:, :, j * qw:(j + 1) * qw],
                    xT_d.ap()[:, :, j * qw:(j + 1) * qw],
                )

            # j0 in halves so the first QKV matmuls start sooner
            nc.sync.dma_start(xT_sb[:, 0:4, 0:qw], xT_d.ap()[:, 0:4, 0:qw])
            nc.sync.dma_start(wqkv_sb[:, 0], wqkv_d.ap()[:, 0])
            nc.sync.dma_start(xT_sb[:, 4:8, 0:qw], xT_d.ap()[:, 4:8, 0:qw])
            nc.sync.dma_start(wqkv_sb[:, 2], wqkv_d.ap()[:, 2])
            for j in range(1, n_j):
                x_piece(j)
            nc.sync.dma_start(wp_sb[:], wp_d.ap())

            # ---- persistent activations ----
            if QK_FP8:
                # pair-packed for DoubleRow scores: [64, 2, n]
                #   partition p: head = p//32, dh = (p%32) + 32*slot
                qT_sb = qkpool.tile([DH, 2, n], fp8)
                kT_sb = qkpool.tile([DH, 2, n], fp8)
            else:
                qT_sb = qkpool.tile([P, n], bf16)
                kT_sb = qkpool.tile([P, n], bf16)
            vT_sb = qkpool.tile([P, n], bf16)       # feature-major V.T (16x)
            # PV V layout: [seq-in-chunk, mc, [V_h0|1|V_h1|1]]
            v_sb = qkpool.tile([P, m_chunks, 2 * (DH + 1)], bf16)
            nc.gpsimd.memset(v_sb[:, :, DH:DH + 1], 1.0)
            nc.gpsimd.memset(v_sb[:, :, 2 * DH + 1:2 * DH + 2], 1.0)

            # ===== PSUM pools (8 banks):
            #   sps:  scores [128, 2*NB] f32 -> 2 banks x SPS_BUFS
            #   pvps: PV acc [128, 2*NB] f32 -> 2 banks, single buffered
            #   auxp: [128, 512] f32 1-bank tiles x2 (QKV acc / transposes / proj)
            sps = tc.alloc_tile_pool(name="sps", bufs=SPS_BUFS, space="PSUM")
            pvps = tc.alloc_tile_pool(name="pvps", bufs=1, space="PSUM")
            auxp = tc.alloc_tile_pool(name="auxp", bufs=2, space="PSUM")

            # ---- engine-dispatched op emitters ----
            def drain(dst, src, bias=None, act_c=612.0, dve_c=658.0, forced=None):
                """PSUM->SBUF copy (+per-partition bias) on ACT or DVE."""
                eng = pick(act_c, dve_c, forced)
                if eng == "act":
                    if bias is None:
                        nc.scalar.copy(dst, src)
                    else:
                        nc.scalar.activation(dst, src, AF.Identity, bias=bias)
                else:
                    if bias is None:
                        nc.vector.tensor_copy(dst, src)
                    else:
                        nc.vector.tensor_scalar(dst, src, bias, None, ALU.add)

            def exp_op(dst_bf16, src_ps):
                if dve_exps[0] < EXP_DVE_MAX:
                    eng = pick(612.0, 658.0)
                else:
                    eng = pick(612.0, 658.0, forced="act")
                if eng == "act":
                    nc.scalar.activation(dst_bf16, src_ps, AF.Exp, scale=SCALE)
                else:
                    dve_exps[0] += 1
                    nc.vector.tensor_scalar(
                        dst_bf16.bitcast(u16), src_ps, float(SCH_A), float(SCH_B),
                        ALU.mult, ALU.add,
                    )

            # ---- PE warmup: burn the p-state ramp while DMAs land ----
            if WARMUP_MM:
                wps = auxp.tile([P, P], bf16, tag="aux", name="warm")
                for i in range(WARMUP_MM):
                    nc.tensor.transpose(wps[:], ident[:], ident[:])

            # ================= QKV ===========================================
            def qkv_j(j, parts):
                jsl = slice(j * qw, (j + 1) * qw)
                for part in parts:
                    ps = auxp.tile([P, qw], f32, tag="aux", name=f"qkv_{part}_{j}")
                    if part == 2:
                        for o in range(D_CHUNKS):
                            nc.tensor.matmul(
                                ps[:], wqkv_sb[:, part, o], xT_sb[:, o, jsl],
                                start=(o == 0), stop=(o == D_CHUNKS - 1),
                            )
                        drain(vT_sb[:, jsl], ps[:],
                              bias=bqkv_sb[:, 2:3])
                        # V.T -> v_sb via PE transpose per m-chunk
                        for mc in range(j * qw // P, (j + 1) * qw // P):
                            tp = auxp.tile([P, P], bf16, tag="aux", name=f"tp_{mc}")
                            nc.tensor.transpose(
                                tp[:], vT_sb[:, mc * P:(mc + 1) * P], ident[:]
                            )
                            # both heads in one strided copy [128, 2, 64]
                            drain(
                                v_sb[:, mc, :].rearrange(
                                    "p (h c) -> p h c", h=2
                                )[:, :, 0:DH],
                                tp[:].rearrange("p (h c) -> p h c", h=2),
                                act_c=292.0, dve_c=258.0,
                            )
                    else:
                        for o in range(D_CHUNKS):
                            nc.tensor.matmul(
                                ps[:], wqkv_sb[:, part, o], xT_sb[:, o, jsl],
                                start=(o == 0), stop=(o == D_CHUNKS - 1),
                            )
                        dst = qT_sb if part == 0 else kT_sb
                        if QK_FP8:
                            # pair-split drains into [64, 2, n] layout
                            for s in range(2):
                                drain(
                                    dst[:, s, jsl],
                                    ps[s * DH:(s + 1) * DH, :],
                                    bias=bqkv_sb[s * DH:(s + 1) * DH, part:part + 1],
                                )
                        else:
                            drain(dst[:, jsl], ps[:], bias=bqkv_sb[:, part:part + 1])

            # ================= attention =====================================
            with (
                tc.tile_pool(name="espool", bufs=ES_BUFS) as espool,
                tc.tile_pool(name="opool", bufs=2) as opool,
                tc.tile_pool(name="outpool", bufs=2) as outpool,
                tc.tile_pool(name="rpool", bufs=2) as rpool,
            ):
                def attn_sc(b, row0, nbb, mcs):
                    nsl = slice(row0, row0 + nbb)
                    out = []
                    for mc in mcs:
                        pair = []
                        for h in range(HPC):
                            s_ps = sps.tile([P, nbb], f32, tag="s",
                                            name=f"s_ps_{b}_{mc}_{h}")
                            if QK_FP8:
                                nc.tensor.matmul(
                                    s_ps[:],
                                    kT_sb[32 * h:32 * (h + 1), :,
                                          mc * P:(mc + 1) * P],
                                    qT_sb[32 * h:32 * (h + 1), :, nsl],
                                    perf_mode=DR,
                                )
                            else:
                                nc.tensor.matmul(
                                    s_ps[:],
                                    kT_sb[h * DH:(h + 1) * DH,
                                          mc * P:(mc + 1) * P],
                                    qT_sb[h * DH:(h + 1) * DH, nsl],
                                    tile_position=(h * DH, 0),
                                )
                            es = espool.tile([P, nbb], bf16, tag="es",
                                             name=f"es_{b}_{mc}_{h}")
                            exp_op(es[:], s_ps[:])
                            pair.append(es)
                        out.append((mc, pair))
                    return out

                def attn_pv(nbb, pvs, mc_es):
                    for mc, pair in mc_es:
                        for h in range(HPC):
                            nc.tensor.matmul(
                                pvs[0:DH + 1, h * nbb:(h + 1) * nbb],
                                v_sb[:, mc, h * (DH + 1):(h + 1) * (DH + 1)],
                                pair[h][:],
                                start=(mc == 0),
                                stop=(mc == m_chunks - 1),
                            )

                def attn_mc_group(b, row0, nbb, pvs, mcs):
                    attn_pv(nbb, pvs, attn_sc(b, row0, nbb, mcs))

                def division(b, nbb, pvs):
                    # O.T = O'.T / rowsum, heads stacked on partitions.
                    # Drain pvs -> SBUF first (frees the PV psum for the next
                    # block after one op), then recip/bcast/mul from the copy.
                    osb = rpool.tile([DH + 1, HPC * nbb], f32, tag="osb",
                                     name=f"osb_{b}")
                    drain(osb[:], pvs[0:DH + 1, :], act_c=1038.0, dve_c=1190.0)
                    rt = rpool.tile([P, HPC * nbb], f32r, tag="recip",
                                    name=f"rt_{b}")
                    oT = opool.tile([P, nbb], bf16, tag="oT", name=f"oT_{b}")
                    with nc.allow_low_precision(reason="f32r recip"):
                        nc.vector.reciprocal(rt[0:1, :], osb[DH:DH + 1, :])
                    load["dve"] += 1100.0
                    for h in range(HPC):
                        hs = slice(h * nbb, (h + 1) * nbb)
                        rb_ps = auxp.tile([P, nbb], f32, tag="aux",
                                          name=f"rb_{b}_{h}")
                        nc.tensor.matmul(rb_ps[0:DH, :], ones_sb[0:1, :],
                                         rt[0:1, hs])
                        nc.vector.tensor_mul(
                            oT[h * DH:(h + 1) * DH, :],
                            osb[0:DH, hs],
                            rb_ps[0:DH, :],
                        )
                        load["dve"] += 658.0
                    return oT

                def proj_piece(b, row0, oT, out_sb, j, dma_now=False):
                    for half in range(D // 512):
                        pp = auxp.tile([P, 512], f32, tag="aux",
                                       name=f"pp_{b}_{j}_{half}")
                        nc.tensor.matmul(
                            pp[:],
                            oT[:, j * P:(j + 1) * P],
                            wp_sb[:, half * 512:(half + 1) * 512],
                        )
                        drain(out_sb[:, j, half * 512:(half + 1) * 512], pp[:])
                    if dma_now:
                        nc.sync.dma_start(
                            out_d.ap()[row0 + j * P:row0 + (j + 1) * P, :],
                            out_sb[:, j],
                        )

                def projection(b, row0, nbb, oT, last=False):
                    nch = nbb // P
                    out_sb = outpool.tile([P, nch, D], f16, tag="out",
                                          name=f"out_{b}")
                    for j in range(nch):
                        proj_piece(b, row0, oT, out_sb, j, dma_now=last)
                    if not last:
                        nc.sync.dma_start(
                            out_d.ap()[row0:row0 + nbb, :].rearrange(
                                "(c p) d -> p c d", p=P
                            ),
                            out_sb[:],
                        )

                blocks = [nb] * n_blocks
                pending = None   # (b, row0, nbb, oT, out_sb) awaiting projection
                row0 = 0
                for b, nbb in enumerate(blocks):
                    pvs = pvps.tile([P, HPC * nbb], f32, tag="pv", name=f"pv_{b}")
                    if b == 0:
                        # fine interleave with QKV j-sweeps
                        mcs_per_j = qw // P
                        for j in range(n_j):
                            qkv_j(j, parts=(1, 0) if j == 0 else (1,))
                            mc_es = attn_sc(b, row0, nbb,
                                            range(j * mcs_per_j,
                                                  (j + 1) * mcs_per_j))
                            qkv_j(j, parts=(2,))
                            attn_pv(nbb, pvs, mc_es)
                        if n_j > 1:
                            qkv_j(1, parts=(0,))
                    else:
                        # pending projection pieces spread between mc groups
                        pj = [None]

                        def proj_next():
                            if pending is None:
                                return
                            pb, prow0, pnbb, poT, pout = pending
                            if pj[0] is None:
                                pj[0] = 0
                            if pj[0] < pnbb // P:
                                proj_piece(pb, prow0, poT, pout, pj[0])
                                pj[0] += 1

                        attn_mc_group(b, row0, nbb, pvs, range(0, 4))
                        proj_next()
                        if b + 1 < n_j:
                            qkv_j(b + 1, parts=(0,))
                        attn_mc_group(b, row0, nbb, pvs, range(4, 8))
                        proj_next()
                        attn_mc_group(b, row0, nbb, pvs, range(8, 12))
                        proj_next()
                        attn_mc_group(b, row0, nbb, pvs, range(12, m_chunks))
                        proj_next()
                        if pending is not None:
                            pb, prow0, pnbb, poT, pout = pending
                            nc.sync.dma_start(
                                out_d.ap()[prow0:prow0 + pnbb, :].rearrange(
                                    "(c p) d -> p c d", p=P
                                ),
                                pout[:],
                            )
                            pending = None
                    oT = division(b, nbb, pvs)
                    out_sb = outpool.tile([P, nbb // P, D], f16, tag="out",
                                          name=f"out_{b}")
                    pending = (b, row0, nbb, oT, out_sb)
                    row0 += nbb
                # final block: pipelined per-j projection + per-j DMA
                pb, prow0, pnbb, poT, pout = pending
                for j in range(pnbb // P):
                    proj_piece(pb, prow0, poT, pout, j, dma_now=True)

            auxp.release()
            pvps.release()
            sps.release()

    nc.compile()
    return nc


def _host_prep(x, w_qkv, b_qkv, w_proj, n=N):
    """Per-core input maps (dtypes match the DRAM tensor declarations)."""
    import ml_dtypes

    fp8 = ml_dtypes.float8_e4m3
    bf = ml_dtypes.bfloat16

    xT = np.ascontiguousarray(
        x.T.reshape(D_CHUNKS, P, n).transpose(1, 0, 2).astype(bf)
    )

    if QK_FP8:
        # q/k feature order for PSUM partition j:
        #   slot s = j // 64, head h = (j % 64) // 32, dh = (j % 32) + 32*s
        j_idx = np.arange(P)
        s_idx, jj = j_idx // DH, j_idx % DH
        qk_feat = (jj // 32) * DH + (jj % 32) + 32 * s_idx
    else:
        qk_feat = np.arange(P)

    in_maps = []
    for c in range(NCORES):
        parts = []
        border = []
        for part in range(3):
            rows = w_qkv[part * D + c * P:part * D + (c + 1) * P, :]
            feat = qk_feat if part != 2 else np.arange(P)
            wperm = rows[feat, :]                          # [128 feat, D]
            # lhsT layout [p, o, feat]
            parts.append(wperm.T.reshape(D_CHUNKS, P, P).transpose(1, 0, 2))
            border.append(b_qkv[part * D + c * P:part * D + (c + 1) * P][feat])
        wqkv = np.ascontiguousarray(np.stack(parts, axis=1).astype(bf))
        bqkv = np.ascontiguousarray(np.stack(border, axis=1).astype(np.float32))
        wpT = np.ascontiguousarray(w_proj[:, c * P:(c + 1) * P].T.astype(bf))
        in_maps.append({"xT": xT, "wqkv": wqkv, "wpT": wpT, "bqkv": bqkv})
    return in_maps


_NC_CACHE = {}


def run(x, w_qkv, b_qkv, w_proj, b_proj, trace=False, n=N, nb=None, **spmd_kwargs):
    from concourse.bass_utils import run_bass_kernel_spmd

    if nb is None:
        nb = NB
    key = (n, nb, SPS_BUFS, ES_BUFS, QK_FP8, EXP_DVE_MAX)
    if key not in _NC_CACHE:
        _NC_CACHE[key] = _build_nc(n=n, nb=nb)
    nc = _NC_CACHE[key]

    in_maps = _host_prep(
        np.asarray(x), np.asarray(w_qkv), np.asarray(b_qkv), np.asarray(w_proj), n=n
    )
    results = run_bass_kernel_spmd(
        nc, in_maps, core_ids=list(range(NCORES)), trace=trace, **spmd_kwargs
    )
    acc = np.zeros((n, D), dtype=np.float64)
    for c in range(NCORES):
        acc += results.results[c]["out_part"].astype(np.float64)
    acc += np.asarray(b_proj).astype(np.float64)
    return acc.astype(np.float32), results


def kernel(x, w_qkv, b_qkv, w_proj, b_proj):
    out, _ = run(x, w_qkv, b_qkv, w_proj, b_proj, trace=False)
    return out
